# revision 14
# baseline (speedup 1.0000x reference)
"""GNN message passing (2x GCNConv + 2 GCN heads + link prediction) on 8 TRN2 cores.

Pipeline (per core, dst-sharded nodes):
  conv1: t1' = (x @ W1) * dinv1        -> bf16 table T1, AllGather
  agg1 : p1 = segsum(T1[src])          -> h1'' = relu(dinv1*p1 + b1)*dinv1
  conv2: t2' = (h1'' @ W2) * dinv1     -> bf16 table T2 [t2'|0], AllGather
  agg2 : p2 = segsum(T2[src])          -> h = dinv1*p2 + b2
  dinv2: from edge_weight segsums      -> h' = h * dinv2
  HT   : [h'|h] bf16 table, AllGather
  agg3 : p3 = segsum(ew * HT.h'[src])  -> aggh = dinv2*p3
  heads: out = log_softmax(aggh @ [W_attr|W_att] + b, per 40)
  lp   : res = sum(HT.h[i] * HT.h[j])

Aggregation: nodes sorted by degree, 128-node batches; messages gathered
row-wise with dma_gather (int16 idx, lo/hi table windows), reduced via
identity-matmul PSUM accumulation; hi-half partials dma_scatter_add'ed
onto the lo-half rows. Host does all integer preprocessing + unpermute.
"""

import os, sys

sys.path.insert(0, "/opt/trn_rl_repo")
os.environ.setdefault("NEURON_SCRATCHPAD_PAGE_SIZE", "512")

import numpy as np
import ml_dtypes

import concourse.tile as tile
import concourse.mybir as mybir
from concourse import bacc
from concourse.bass import AP
from concourse.bass_utils import run_bass_kernel_spmd
from concourse.masks import make_identity

# --- patch: make Tile's DMASW sem-lane choice respect SWDGE queue_num ---
# Runtime locks each DMASW sem to one SWDGE queue; Tile's round-robin lane
# assignment ignores queue_num.  Map queue q -> lanes {2q, 2q+1}.
import concourse.tile_sem_assignment as _tsa
import concourse.bass_isa as _bass_isa
from concourse.tile_scheduler import DMAInst as _DMAInst

_QSTR = {"qPoolDynamic": 0, "qPoolDynamic1": 1, "qPoolDynamic2": 2,
         "qPoolDynamic3": 3}

if not getattr(_tsa.TileClockTick, "_gnn_patched", False):
    _orig_assign_tick = _tsa.TileClockTick._assign_tick

    def _assign_tick(self, inst):
        if (isinstance(inst, _DMAInst)
                and inst.engine == mybir.EngineType.Pool
                and not isinstance(inst, _bass_isa.UserSyncedRemoteDMADescs)):
            qn = getattr(inst, "queue_num", None)
            if qn is None:
                qn = _QSTR.get(getattr(inst, "queue", None), 0)
            tog = getattr(self, "_gnn_qtoggle", None)
            if tog is None:
                tog = self._gnn_qtoggle = {}
            t = tog.get(qn, 0)
            tog[qn] = t ^ 1
            if self.swdge_sem_count >= 8:
                self.next_sw_dma_idx = (2 * qn + t) % self.swdge_sem_count
        return _orig_assign_tick(self, inst)

    _tsa.TileClockTick._assign_tick = _assign_tick
    _tsa.TileClockTick._gnn_patched = True
# --- end patch ---

F32 = mybir.dt.float32
F32R = mybir.dt.float32r
BF16 = mybir.dt.bfloat16
I32 = mybir.dt.int32
I16 = mybir.dt.int16
AluOp = mybir.AluOpType
ActF = mybir.ActivationFunctionType
AxX = mybir.AxisListType.X


class CFG:
    def __init__(self, n_nodes=50000, d_feat=512, h1=128, h2=64, n_classes=40,
                 e_lp=250000, nc=8):
        self.N = n_nodes
        self.D_FEAT = d_feat
        self.H1 = h1
        self.H2 = h2
        self.NCLS = n_classes
        self.DCAT = 2 * n_classes
        self.NC = nc
        self.NSLICE = n_nodes // nc
        self.NB = (self.NSLICE + 127) // 128
        self.NP = self.NB * 128
        self.ZPAD = 16
        self.TBL_ROWS = self.ZPAD + nc * self.NP + 16
        self.HI_BASE = max(self.TBL_ROWS - 32768, 0)
        self.LO_SPLIT = self.ZPAD + (nc // 2) * self.NP
        self.ZR_LO = 0
        self.ZR_HI = self.ZPAD + nc * self.NP
        self.LP_PER_CORE = 2 * e_lp // nc
        assert self.LO_SPLIT - 1 < 32768
        assert (self.TBL_ROWS - 1) - self.HI_BASE < 32768


CFG_FULL = CFG()


# =================== host preprocessing ===================

def _wrap16(idx_lin):
    """dma_gather idx layout: linear pos i -> [i%16, i//16]; tiled to 128 rows."""
    n = idx_lin.shape[0]
    assert n % 16 == 0
    a = np.asarray(idx_lin, np.int16).reshape(n // 16, 16).T.copy()
    return np.tile(a, (8, 1))


def _build_struct(cfg, dst_local, src_pos, ew):
    """One edge set on one core -> dict with per-half layouts."""
    out = {}
    for half in (0, 1):
        if half == 0:
            m = src_pos < cfg.LO_SPLIT
            rel = src_pos[m]
            zr = cfg.ZR_LO
        else:
            m = src_pos >= cfg.LO_SPLIT
            rel = src_pos[m] - cfg.HI_BASE
            zr = cfg.ZR_HI - cfg.HI_BASE
        d = dst_local[m]
        w = ew[m] if ew is not None else None
        deg = np.bincount(d, minlength=cfg.NSLICE)
        order = np.argsort(-deg, kind="stable")
        out[half] = dict(deg=deg, order=order, dst=d, rel=rel, ew=w, zr=zr)
    return out


def _caps_of(cfg, deg, order):
    caps = []
    for b in range(cfg.NB):
        sl = order[b * 128:(b + 1) * 128]
        sl = sl[sl < cfg.NSLICE]
        caps.append(max(int(deg[sl].max()) if sl.size else 0, 1))
    return caps


def _fill_struct(cfg, st, caps_pair):
    """Build idx16 / ew-slot arrays for both halves + row maps."""
    res = dict(idx16=[], ewslots=[])
    for half in (0, 1):
        h = st[half]
        caps = caps_pair[half]
        order = h["order"]
        es = np.argsort(h["dst"], kind="stable")
        dst_s = h["dst"][es]
        rel_s = h["rel"][es]
        ew_s = h["ew"][es] if h["ew"] is not None else None
        starts = np.searchsorted(dst_s, np.arange(cfg.NSLICE))
        ends = np.searchsorted(dst_s, np.arange(cfg.NSLICE) + 1)
        tot = int(np.sum(caps) * 128)
        idx_lin = np.full(tot, h["zr"], np.int64)
        ew_lin = np.zeros(tot, np.float32) if ew_s is not None else None
        rowpos = np.full(cfg.NSLICE, -1, np.int64)
        off = 0
        for b in range(cfg.NB):
            cap = caps[b]
            lanes = order[b * 128:(b + 1) * 128]
            for p, node in enumerate(lanes):
                if node >= cfg.NSLICE:
                    continue
                rowpos[node] = b * 128 + p
                s, e = starts[node], ends[node]
                cnt = e - s
                assert cnt <= cap
                t = np.arange(cnt)
                idx_lin[off + t * 128 + p] = rel_s[s:e]
                if ew_lin is not None:
                    ew_lin[off + t * 128 + p] = ew_s[s:e]
            off += cap * 128
        res["idx16"].append(_wrap16(idx_lin))
        res["ewslots"].append(
            ew_lin.reshape(-1, 128).T.copy() if ew_lin is not None else None)
        res[f"order{half}"] = order
        res[f"rowpos{half}"] = rowpos
        res[f"deg{half}"] = h["deg"]
    # hi row -> lo row of same node; pads -> NP (dump row)
    smap = np.full(cfg.NP, cfg.NP, np.int64)
    for r in range(cfg.NP):
        node = res["order1"][r] if r < cfg.NSLICE else -1
        if node >= 0:
            smap[r] = res["rowpos0"][node]
    res["scatter_map"] = smap
    # lo row -> hi row of same node; pads -> NP (zero row in deg buffer)
    lmap = np.full(cfg.NP, cfg.NP, np.int64)
    for r in range(cfg.NP):
        node = res["order0"][r] if r < cfg.NSLICE else -1
        if node >= 0:
            lmap[r] = res["rowpos1"][node]
    res["lo_to_hi"] = lmap
    return res


def preprocess(cfg, x, train_pos_edge_index, edge_index, edge_weight,
               pos_edge_index, neg_edge_index):
    N, NC, NSLICE, NP, NB = cfg.N, cfg.NC, cfg.NSLICE, cfg.NP, cfg.NB
    tpe = np.asarray(train_pos_edge_index, np.int64)
    ei = np.asarray(edge_index, np.int64)
    ew = np.asarray(edge_weight, np.float32)
    loops = np.arange(N, dtype=np.int64)

    srcA = np.concatenate([tpe[0] % N, loops])
    dstA = np.concatenate([tpe[1] % N, loops])
    srcB = np.concatenate([ei[0] % N, loops])
    dstB = np.concatenate([ei[1] % N, loops])
    ewB = np.concatenate([ew, np.ones(N, np.float32)])

    # π table order = lo-half-degree sorted order of structure A per core.
    # lo/hi split by src core (< NC/2), independent of fine row.
    ordA = []
    for c in range(NC):
        lo, hi = c * NSLICE, (c + 1) * NSLICE
        m = (dstA >= lo) & (dstA < hi)
        s, d = srcA[m], dstA[m] - lo
        deg_lo = np.bincount(d[(s // NSLICE) < NC // 2], minlength=NSLICE)
        ordA.append(np.argsort(-deg_lo, kind="stable"))

    node_row = np.zeros(N, np.int64)
    for c in range(NC):
        rows = np.full(NSLICE, -1, np.int64)
        rows[ordA[c]] = np.arange(NSLICE)
        node_row[c * NSLICE:(c + 1) * NSLICE] = cfg.ZPAD + c * NP + rows

    posA = node_row[srcA]
    posB = node_row[srcB]

    structsA, structsB = [], []
    for c in range(NC):
        lo, hi = c * NSLICE, (c + 1) * NSLICE
        mA = (dstA >= lo) & (dstA < hi)
        structsA.append(_build_struct(cfg, dstA[mA] - lo, posA[mA], None))
        mB = (dstB >= lo) & (dstB < hi)
        structsB.append(_build_struct(cfg, dstB[mB] - lo, posB[mB], ewB[mB]))

    capsA = [[max(_caps_of(cfg, structsA[c][h]["deg"], structsA[c][h]["order"])[b]
                  for c in range(NC)) for b in range(NB)] for h in (0, 1)]
    capsB = [[max(_caps_of(cfg, structsB[c][h]["deg"], structsB[c][h]["order"])[b]
                  for c in range(NC)) for b in range(NB)] for h in (0, 1)]

    fillsA = [_fill_struct(cfg, structsA[c], capsA) for c in range(NC)]
    fillsB = [_fill_struct(cfg, structsB[c], capsB) for c in range(NC)]
    for c in range(NC):
        assert np.array_equal(fillsA[c]["order0"], ordA[c])

    degA_full = np.bincount(dstA, minlength=N)
    dinv1 = (degA_full.astype(np.float64) ** -0.5).astype(np.float32)

    # link prediction groups
    tot_i = np.concatenate([np.asarray(pos_edge_index[0]),
                            np.asarray(neg_edge_index[0])]).astype(np.int64) % N
    tot_j = np.concatenate([np.asarray(pos_edge_index[1]),
                            np.asarray(neg_edge_index[1])]).astype(np.int64) % N
    # reference: x_j = h[total[0]], x_i = h[total[1]]; res = sum(x_i * x_j)
    pos_i = node_row[tot_i]
    pos_j = node_row[tot_j]
    LPB = cfg.LP_PER_CORE
    groups = []
    for c in range(NC):
        sl = slice(c * LPB, (c + 1) * LPB)
        g = (pos_i[sl] >= cfg.LO_SPLIT) * 2 + (pos_j[sl] >= cfg.LO_SPLIT)
        groups.append(g)
    gcnt = np.array([[int((groups[c] == k).sum()) for k in range(4)]
                     for c in range(NC)])
    gpad = [int(-(-gcnt[:, k].max() // 128) * 128) for k in range(4)]
    # chunk boundaries at multiples of 128 within each group
    lp_tot = sum(gpad)
    lp_info = []  # (chunk_col0, ncols, base_i_is_hi, base_j_is_hi) per chunk
    col = 0
    LPC = 8
    for k in range(4):
        ncols = gpad[k] // 128
        c0 = 0
        while c0 < ncols:
            n = min(LPC, ncols - c0)
            lp_info.append((col + c0, n, k >= 2, k % 2 == 1))
            c0 += n
        col += ncols

    in_maps = []
    for c in range(NC):
        fA, fB = fillsA[c], fillsB[c]
        order = ordA[c]
        lo, hi = c * NSLICE, (c + 1) * NSLICE
        xp = np.zeros((NP, cfg.D_FEAT), np.float32)
        xp[:NSLICE] = np.asarray(x[lo:hi], np.float32)[order]
        d1 = np.zeros(NP, np.float32)
        d1[:NSLICE] = dinv1[lo:hi][order]

        sl = slice(c * LPB, (c + 1) * LPB)
        g = groups[c]
        idx_i = np.zeros(lp_tot, np.int64)
        idx_j = np.zeros(lp_tot, np.int64)
        perm = np.full(lp_tot, -1, np.int64)
        off = 0
        pi, pj = pos_i[sl], pos_j[sl]
        for k in range(4):
            eids = np.where(g == k)[0]
            n = eids.shape[0]
            bi = cfg.HI_BASE if k >= 2 else 0
            bj = cfg.HI_BASE if (k % 2) else 0
            zi = (cfg.ZR_HI - cfg.HI_BASE) if k >= 2 else cfg.ZR_LO
            zj = (cfg.ZR_HI - cfg.HI_BASE) if (k % 2) else cfg.ZR_LO
            idx_i[off:off + gpad[k]] = zi
            idx_j[off:off + gpad[k]] = zj
            idx_i[off:off + n] = pi[eids] - bi
            idx_j[off:off + n] = pj[eids] - bj
            perm[off:off + n] = eids
            off += gpad[k]

        im = {
            "xT": np.ascontiguousarray(xp.T),
            "d1_plane": np.ascontiguousarray(d1.reshape(NB, 128).T),
            "idxA_lo": fA["idx16"][0], "idxA_hi": fA["idx16"][1],
            "idxB_lo": fB["idx16"][0], "idxB_hi": fB["idx16"][1],
            "ewB_lo": fB["ewslots"][0], "ewB_hi": fB["ewslots"][1],
            "scatA": _wrap16(fA["scatter_map"]),
            "scatB": _wrap16(fB["scatter_map"]),
            "mapAtoB": _wrap16(np.array(
                [fB["rowpos0"][order[r]] if r < NSLICE else NP
                 for r in range(NP)], np.int64)),
            "mapLoHiB": _wrap16(fB["lo_to_hi"]),
            "lp_i": _wrap16(idx_i), "lp_j": _wrap16(idx_j),
        }
        in_maps.append(im)

    meta = dict(capsA=capsA, capsB=capsB, lp_tot=lp_tot, lp_info=lp_info,
                ordA=ordA, ordB=[fillsB[c]["order0"] for c in range(NC)],
                lp_perm=[None] * NC, node_row=node_row)
    # store perms for unpermute
    meta["lp_perm"] = []
    for c in range(NC):
        sl = slice(c * LPB, (c + 1) * LPB)
        g = groups[c]
        perm = np.full(lp_tot, -1, np.int64)
        off = 0
        for k in range(4):
            eids = np.where(g == k)[0]
            perm[off:off + eids.shape[0]] = eids
            off += gpad[k]
        meta["lp_perm"].append(perm)
    return in_maps, meta


# =================== device program ===================

def build_program(cfg, capsA, capsB, lp_tot, lp_info):
    NP, NB, NC = cfg.NP, cfg.NB, cfg.NC
    H1, H2, DCAT, D_FEAT = cfg.H1, cfg.H2, cfg.DCAT, cfg.D_FEAT
    TBL_ROWS, ZPAD, HI_BASE = cfg.TBL_ROWS, cfg.ZPAD, cfg.HI_BASE
    KD = D_FEAT // 128

    nc = bacc.Bacc("TRN2", target_bir_lowering=False, debug=False,
                   num_devices=NC, num_swdge_queues=4)

    capA_lo, capA_hi = capsA
    capB_lo, capB_hi = capsB
    totA_lo, totA_hi = sum(capA_lo) * 128, sum(capA_hi) * 128
    totB_lo, totB_hi = sum(capB_lo) * 128, sum(capB_hi) * 128

    def dram_in(name, shape, dt):
        return nc.dram_tensor(name, shape, dt, kind="ExternalInput")

    xT_in = dram_in("xT", [D_FEAT, NP], F32R)
    d1_in = dram_in("d1_plane", [128, NB], F32)
    idxA_lo_in = dram_in("idxA_lo", [128, totA_lo // 16], I16)
    idxA_hi_in = dram_in("idxA_hi", [128, totA_hi // 16], I16)
    idxB_lo_in = dram_in("idxB_lo", [128, totB_lo // 16], I16)
    idxB_hi_in = dram_in("idxB_hi", [128, totB_hi // 16], I16)
    ewB_lo_in = dram_in("ewB_lo", [128, totB_lo // 128], F32)
    ewB_hi_in = dram_in("ewB_hi", [128, totB_hi // 128], F32)
    scatA_in = dram_in("scatA", [128, NP // 16], I16)
    scatB_in = dram_in("scatB", [128, NP // 16], I16)
    mapAB_in = dram_in("mapAtoB", [128, NP // 16], I16)
    mapLH_in = dram_in("mapLoHiB", [128, NP // 16], I16)
    lp_i_in = dram_in("lp_i", [128, lp_tot // 16], I16)
    lp_j_in = dram_in("lp_j", [128, lp_tot // 16], I16)
    W1_in = dram_in("W1", [D_FEAT, H1], F32R)
    W2_in = dram_in("W2", [128, H2], BF16)
    Wcat_in = dram_in("Wcat", [H2, DCAT], F32)
    b1r_in = dram_in("b1_rep", [128, H1], F32)
    b2r_in = dram_in("b2_rep", [128, H2], F32)
    bcatr_in = dram_in("bcat_rep", [128, DCAT], F32)

    attr_out = nc.dram_tensor("attr_att", [NP, DCAT], F32, kind="ExternalOutput")
    res_out = nc.dram_tensor("res", [128, lp_tot // 128], F32, kind="ExternalOutput")
    DEBUG = os.environ.get("GNN_DEBUG") == "1"
    if DEBUG:
        dbg_aggh = nc.dram_tensor("dbg_aggh", [NP, H2], F32, kind="ExternalOutput")
        dbg_gi = nc.dram_tensor("dbg_gi", [128, 8 * 128], F32, kind="ExternalOutput")
        dbg_gj = nc.dram_tensor("dbg_gj", [128, 8 * 128], F32, kind="ExternalOutput")
        dbg_z = nc.dram_tensor("dbg_z", [NP, DCAT], F32, kind="ExternalOutput")

    qctr = [0]
    def q():
        qctr[0] = (qctr[0] + 1) % 4
        return qctr[0]

    with tile.TileContext(nc) as tc:
        with (
            tc.tile_pool(name="sbc", bufs=1) as sbc,
            tc.tile_pool(name="sbw", bufs=2) as sbw,
            tc.tile_pool(name="sbi", bufs=2) as sbi,
            tc.tile_pool(name="sbg", bufs=3) as sbg,
            tc.tile_pool(name="ps", bufs=2, space="PSUM") as ps,
            tc.tile_pool(name="ps1", bufs=2, space="PSUM") as ps1,
            tc.tile_pool(name="dram", bufs=1, space="DRAM") as dram,
        ):
            # ---- constants ----
            identb = sbc.tile([128, 128], BF16)
            make_identity(nc, identb[:])
            identf = sbc.tile([128, 128], F32)
            make_identity(nc, identf[:])

            def load_const(name, inp, shape, dt):
                t = sbc.tile(shape, dt, tag=name)
                nc.sync.dma_start(t[:], inp[:])
                return t

            W1_sb = sbc.tile([128, KD, H1], F32R)
            nc.sync.dma_start(W1_sb[:], W1_in[:].rearrange("(a p) o -> p a o", p=128))
            W2_sb = load_const("W2", W2_in, [128, H2], BF16)
            Wcat_sb = load_const("Wcat", Wcat_in, [H2, DCAT], F32)
            b1r = load_const("b1r", b1r_in, [128, H1], F32)
            b2r = load_const("b2r", b2r_in, [128, H2], F32)
            bcatr = load_const("bcatr", bcatr_in, [128, DCAT], F32)
            d1p = load_const("d1p", d1_in, [128, NB], F32)
            IDXW = max(totA_lo, totA_hi, totB_lo, totB_hi) // 16
            ewB_lo = load_const("eBl", ewB_lo_in, [128, totB_lo // 128], F32)
            ewB_hi = load_const("eBh", ewB_hi_in, [128, totB_hi // 128], F32)
            scatA = load_const("sA", scatA_in, [128, NP // 16], I16)
            scatB = load_const("sB", scatB_in, [128, NP // 16], I16)
            mapAB = load_const("mAB", mapAB_in, [128, NP // 16], I16)
            mapLH = load_const("mLH", mapLH_in, [128, NP // 16], I16)


            zrow = sbc.tile([16, 128], BF16)
            nc.vector.memset(zrow[:], 0.0)

            # ---- DRAM buffers ----
            T1 = dram.tile([TBL_ROWS, 128], BF16)
            T2 = dram.tile([TBL_ROWS, 128], BF16)
            HT = dram.tile([TBL_ROWS, 128], BF16)
            t1_slice = dram.tile([NP, 128], BF16)
            t2_slice = dram.tile([NP, 128], BF16)
            ht_slice = dram.tile([NP, 128], BF16)
            p1 = dram.tile([NP + 16, H1], F32)
            p2 = dram.tile([NP + 16, H2], F32)
            p3 = dram.tile([NP + 16, H2], F32)
            deg_hi_d = dram.tile([NP + 16, 64], F32)

            for T in (T1, T2, HT):
                nc.sync.dma_start(T[0:16, :], zrow[:])
                nc.sync.dma_start(T[cfg.ZR_HI:cfg.ZR_HI + 16, :], zrow[:])

            # ---------- conv1 ----------
            for j in range((NP + 511) // 512):
                nj = min(512, NP - j * 512)
                nt = nj // 128
                acc = ps.tile([128, 512], F32, tag="c1")
                for k in range(KD):
                    xt = sbw.tile([128, 512], F32R, tag="xt")
                    nc.sync.dma_start(
                        xt[:, 0:nj], xT_in[k * 128:(k + 1) * 128,
                                           j * 512:j * 512 + nj])
                    nc.tensor.matmul(out=acc[:, 0:nj], lhsT=W1_sb[:, k, :],
                                     rhs=xt[:, 0:nj],
                                     start=(k == 0), stop=(k == KD - 1))
                ev = sbw.tile([128, 512], F32, tag="c1e")
                nc.scalar.copy(ev[:, 0:nj], acc[:, 0:nj])
                rows = sbw.tile([128, 4, 128], BF16, tag="c1r")
                for t in range(nt):
                    tp = ps1.tile([128, 128], F32, tag="tp")
                    nc.tensor.transpose(out=tp[:], in_=ev[:, t * 128:(t + 1) * 128],
                                        identity=identf[:])
                    bidx = j * 4 + t
                    nc.vector.tensor_scalar_mul(rows[:, t, :], tp[:],
                                                d1p[:, bidx:bidx + 1])
                nc.sync.dma_start(
                    t1_slice[j * 512:j * 512 + nj, :]
                        .rearrange("(a p) d -> p a d", p=128),
                    rows[:, 0:nt, :])

            nc.gpsimd.collective_compute(
                "AllGather", AluOp.bypass, replica_groups=[list(range(NC))],
                ins=[t1_slice[:]], outs=[T1[ZPAD:ZPAD + NC * NP, :]])

            # ---------- generic aggregation ----------
            def aggregate(TBL, idx_pair, caps_pair, ew_pair, scat, partial, D,
                          tag):
                for half in (0, 1):
                    caps = caps_pair[half]
                    idx_dram = idx_pair[half]
                    tot16 = sum(caps) * 8
                    idx = sbi.tile([128, IDXW], I16, tag="idx")
                    nc.sync.dma_start(idx[:, 0:tot16], idx_dram[:])
                    ew = ew_pair[half] if ew_pair else None
                    base = 0 if half == 0 else HI_BASE
                    off = 0
                    for c0 in range(0, NB, 8):
                        c1 = min(c0 + 8, NB)
                        stage = sbw.tile([128, 8, D], F32, tag=f"st{tag}")
                        for b in range(c0, c1):
                            cap = caps[b]
                            g = sbg.tile([128, cap, 128], BF16, tag="g")
                            nc.gpsimd.dma_gather(
                                out_ap=g[:], in_ap=TBL[base:, :],
                                idxs_ap=idx[:, off // 16:(off + cap * 128) // 16],
                                num_idxs=cap * 128, num_idxs_reg=cap * 128,
                                elem_size=128, single_packet=False, queue_num=q())
                            if ew is not None:
                                gs = sbg.tile([128, cap, H2], BF16, tag="gs")
                                ew_sl = ew[:, off // 128:off // 128 + cap]
                                ew_b = AP(ew_sl.tensor, ew_sl.offset,
                                          ew_sl.ap + [[0, H2]])
                                nc.vector.tensor_tensor(
                                    out=gs[:], in0=g[:, :, 0:H2], in1=ew_b,
                                    op=AluOp.mult)
                                rhs_view, rsl = gs, slice(0, H2)
                            else:
                                rhs_view, rsl = g, slice(0, D)
                            acc = ps.tile([128, D], F32, tag="acc")
                            for t in range(cap):
                                nc.tensor.matmul(
                                    out=acc[:], lhsT=identb[:],
                                    rhs=rhs_view[:, t, rsl],
                                    start=(t == 0), stop=(t == cap - 1))
                            nc.scalar.copy(stage[:, b - c0, :], acc[:])
                            off += cap * 128
                        n = c1 - c0
                        if half == 0:
                            nc.sync.dma_start(
                                partial[c0 * 128:c1 * 128, :]
                                    .rearrange("(a p) d -> p a d", p=128),
                                stage[:, 0:n, :])
                        else:
                            nc.gpsimd.dma_scatter_add(
                                out_ap=partial[:, :],
                                in_ap=stage[:, 0:n, :],
                                idxs_ap=scat[:, c0 * 8:c1 * 8],
                                num_idxs=n * 128, num_idxs_reg=n * 128,
                                elem_size=D, single_packet=False,
                                queue_num=q())

            # ---------- agg1 + finalize -> h1T ----------
            aggregate(T1, (idxA_lo_in, idxA_hi_in), (capA_lo, capA_hi), None, scatA,
                      p1, H1, "1")

            h1T = sbc.tile([128, NP], BF16)
            for b in range(NB):
                pr = sbw.tile([128, H1], F32, tag="f1")
                nc.sync.dma_start(pr[:], p1[b * 128:(b + 1) * 128, :])
                v = sbw.tile([128, H1], F32, tag="f1v")
                nc.scalar.mul(v[:], pr[:], d1p[:, b:b + 1])
                nc.vector.tensor_tensor(out=v[:], in0=v[:], in1=b1r[:],
                                        op=AluOp.add)
                dcol = d1p[:, b:b + 1]
                nc.vector.scalar_tensor_tensor(
                    out=v[:], in0=v[:], scalar=0.0,
                    in1=dcol.to_broadcast([128, H1]),
                    op0=AluOp.max, op1=AluOp.mult)
                tp = ps1.tile([128, 128], F32, tag="tp")
                nc.tensor.transpose(out=tp[:], in_=v[:], identity=identf[:])
                nc.vector.tensor_copy(h1T[:, b * 128:(b + 1) * 128], tp[:])

            # ---------- conv2 ----------
            for j in range(NB):
                acc = ps1.tile([128, H2], F32, tag="sm")
                nc.tensor.matmul(out=acc[:], lhsT=h1T[:, j * 128:(j + 1) * 128],
                                 rhs=W2_sb[:], start=True, stop=True)
                rows = sbw.tile([128, 128], BF16, tag="c2r")
                nc.vector.memset(rows[:, H2:128], 0.0)
                nc.vector.tensor_copy(rows[:, 0:H2], acc[:])
                nc.sync.dma_start(t2_slice[j * 128:(j + 1) * 128, :], rows[:])

            nc.gpsimd.collective_compute(
                "AllGather", AluOp.bypass, replica_groups=[list(range(NC))],
                ins=[t2_slice[:]], outs=[T2[ZPAD:ZPAD + NC * NP, :]])

            # ---------- agg2 ----------
            aggregate(T2, (idxA_lo_in, idxA_hi_in), (capA_lo, capA_hi), None, scatA,
                      p2, H2, "2")

            # ---------- dinv2 ----------
            deg_lo_p = sbw.tile([128, NB], F32, tag="dgl")
            deg_hi_p = sbw.tile([128, NB], F32, tag="dgh")
            for half, (caps, ewt, plane) in enumerate((
                    (capB_lo, ewB_lo, deg_lo_p), (capB_hi, ewB_hi, deg_hi_p))):
                off = 0
                for b in range(NB):
                    cap = caps[b]
                    nc.vector.reduce_sum(plane[:, b:b + 1], ewt[:, off:off + cap],
                                         axis=AxX)
                    off += cap
            # hi plane -> DRAM rows (π_hiB) widened to 64 cols, plus zero pads
            degx = sbw.tile([128, NB, 64], F32, tag="degx")
            dsl = deg_hi_p[:]
            nc.vector.tensor_copy(degx[:], AP(dsl.tensor, dsl.offset,
                                              dsl.ap + [[0, 64]]))
            nc.sync.dma_start(
                deg_hi_d[0:NP, :].rearrange("(a p) d -> p a d", p=128), degx[:])
            z1 = sbw.tile([16, 64], F32, tag="z1")
            nc.vector.memset(z1[:], 0.0)
            nc.sync.dma_start(deg_hi_d[NP:NP + 16, :], z1[:])
            # gather hi-deg into lo order (256B rows, int16 map)
            deg_hi_lo = sbw.tile([128, NB, 64], F32, tag="dghl")
            nc.gpsimd.dma_gather(
                out_ap=deg_hi_lo[:], in_ap=deg_hi_d[:], idxs_ap=mapLH[:],
                num_idxs=NP, num_idxs_reg=NP, elem_size=64,
                single_packet=False, queue_num=q())
            deg2 = sbw.tile([128, NB], F32, tag="dg2")
            nc.vector.tensor_tensor(out=deg2[:], in0=deg_lo_p[:],
                                    in1=deg_hi_lo[:, :, 0],
                                    op=AluOp.add)
            nc.vector.tensor_scalar_max(deg2[:], deg2[:], 1e-12)
            sq = sbw.tile([128, NB], F32, tag="sq")
            nc.scalar.sqrt(sq[:], deg2[:])
            dinv2 = sbc.tile([128, NB], F32)
            nc.vector.reciprocal(dinv2[:], sq[:])
            # dinv2 to DRAM (π_loB rows, widened) then map to ord1 rows
            d2B = dram.tile([NP + 16, 64], F32)
            d2x = sbw.tile([128, NB, 64], F32, tag="degx")
            dsl2 = dinv2[:]
            nc.vector.tensor_copy(d2x[:], AP(dsl2.tensor, dsl2.offset,
                                             dsl2.ap + [[0, 64]]))
            nc.sync.dma_start(
                d2B[0:NP, :].rearrange("(a p) d -> p a d", p=128), d2x[:])
            nc.sync.dma_start(d2B[NP:NP + 16, :], z1[:])
            dinv2_o1 = sbc.tile([128, NB, 64], F32)
            nc.gpsimd.dma_gather(
                out_ap=dinv2_o1[:], in_ap=d2B[:], idxs_ap=mapAB[:],
                num_idxs=NP, num_idxs_reg=NP, elem_size=64,
                single_packet=False, queue_num=q())

            # ---------- finalize h / h' -> HT ----------
            for b in range(NB):
                pr = sbw.tile([128, H2], F32, tag="f2")
                nc.sync.dma_start(pr[:], p2[b * 128:(b + 1) * 128, :])
                hraw = sbw.tile([128, H2], F32, tag="f2h")
                nc.scalar.mul(hraw[:], pr[:], d1p[:, b:b + 1])
                nc.vector.tensor_tensor(out=hraw[:], in0=hraw[:], in1=b2r[:],
                                        op=AluOp.add)
                rows = sbw.tile([128, 128], BF16, tag="f2r")
                nc.vector.tensor_scalar_mul(rows[:, 0:H2], hraw[:],
                                            dinv2_o1[:, b, 0:1])
                nc.vector.tensor_copy(rows[:, H2:128], hraw[:])
                nc.sync.dma_start(ht_slice[b * 128:(b + 1) * 128, :], rows[:])

            nc.gpsimd.collective_compute(
                "AllGather", AluOp.bypass, replica_groups=[list(range(NC))],
                ins=[ht_slice[:]], outs=[HT[ZPAD:ZPAD + NC * NP, :]])

            # ---------- agg3 ----------
            aggregate(HT, (idxB_lo_in, idxB_hi_in), (capB_lo, capB_hi),
                      (ewB_lo, ewB_hi), scatB, p3, H2, "3")

            # ---------- heads + log_softmax ----------
            for c0 in range(0, NB, 8):
                c1 = min(c0 + 8, NB)
                n = c1 - c0
                z = sbw.tile([128, 8, DCAT], F32, tag="z")
                for b in range(c0, c1):
                    pr = sbw.tile([128, H2], F32, tag="f3")
                    nc.sync.dma_start(pr[:], p3[b * 128:(b + 1) * 128, :])
                    v = sbw.tile([128, H2], F32, tag="f3v")
                    nc.vector.tensor_scalar_mul(v[:], pr[:],
                                                dinv2[:, b:b + 1])
                    if DEBUG:
                        nc.sync.dma_start(dbg_aggh[b * 128:(b + 1) * 128, :], v[:])
                    tp = ps1.tile([H2, 128], F32, tag="tp")
                    nc.tensor.transpose(out=tp[:], in_=v[:], identity=identf[:])
                    aggT = sbw.tile([H2, 128], F32, tag="aggT")
                    nc.scalar.copy(aggT[:], tp[:])
                    om = ps1.tile([128, DCAT], F32, tag="sm")
                    nc.tensor.matmul(out=om[:], lhsT=aggT[:], rhs=Wcat_sb[:],
                                     start=True, stop=True)
                    nc.vector.tensor_tensor(out=z[:, b - c0, :], in0=om[:],
                                            in1=bcatr[:], op=AluOp.add)
                    if DEBUG:
                        nc.sync.dma_start(dbg_z[b * 128:(b + 1) * 128, :],
                                          z[:, b - c0, :])
                # log softmax over each 40-class group
                zv = z[:, 0:n, :]
                z4 = AP(zv.tensor, zv.offset,
                        [zv.ap[0], zv.ap[1], [cfg.NCLS, 2], [1, cfg.NCLS]])
                mx = sbw.tile([128, 8, 2], F32, tag="mx")
                nc.vector.reduce_max(mx[:, 0:n, :], z4, axis=AxX)
                mxs = mx[:, 0:n, :]
                mxb = AP(mxs.tensor, mxs.offset, mxs.ap + [[0, cfg.NCLS]])
                zc = sbw.tile([128, 8, 2, cfg.NCLS], F32, tag="zc")
                nc.vector.tensor_tensor(out=zc[:, 0:n, :, :], in0=z4, in1=mxb,
                                        op=AluOp.subtract)
                e = sbw.tile([128, 8, 2, cfg.NCLS], F32, tag="e")
                nc.scalar.activation(e[:, 0:n, :, :], zc[:, 0:n, :, :], ActF.Exp)
                s = sbw.tile([128, 8, 2], F32, tag="s")
                nc.vector.reduce_sum(s[:, 0:n, :], e[:, 0:n, :, :], axis=AxX)
                ls = sbw.tile([128, 8, 2], F32, tag="ls")
                nc.scalar.activation(ls[:, 0:n, :], s[:, 0:n, :], ActF.Ln)
                lss = ls[:, 0:n, :]
                lsb = AP(lss.tensor, lss.offset, lss.ap + [[0, cfg.NCLS]])
                oo = sbw.tile([128, 8, DCAT], F32, tag="oo")
                ov = oo[:, 0:n, :]
                o4 = AP(ov.tensor, ov.offset,
                        [ov.ap[0], ov.ap[1], [cfg.NCLS, 2], [1, cfg.NCLS]])
                nc.vector.tensor_tensor(out=o4, in0=zc[:, 0:n, :, :], in1=lsb,
                                        op=AluOp.subtract)
                nc.sync.dma_start(
                    attr_out[c0 * 128:c1 * 128, :]
                        .rearrange("(a p) d -> p a d", p=128),
                    oo[:, 0:n, :])

            # ---------- link prediction ----------
            for (col0, ncols, i_hi, j_hi) in lp_info:
                gi = sbg.tile([128, 8, 128], BF16, tag="gi")
                gj = sbg.tile([128, 8, 128], BF16, tag="gj")
                lpi_t = sbi.tile([128, 8 * 8], I16, tag="lpi")
                nc.sync.dma_start(lpi_t[:, 0:ncols * 8],
                                  lp_i_in[:, col0 * 8:(col0 + ncols) * 8])
                lpj_t = sbi.tile([128, 8 * 8], I16, tag="lpj")
                nc.sync.dma_start(lpj_t[:, 0:ncols * 8],
                                  lp_j_in[:, col0 * 8:(col0 + ncols) * 8])
                bi = HI_BASE if i_hi else 0
                bj = HI_BASE if j_hi else 0
                nc.gpsimd.dma_gather(
                    out_ap=gi[:, 0:ncols, :], in_ap=HT[bi:, :],
                    idxs_ap=lpi_t[:, 0:ncols * 8],
                    num_idxs=ncols * 128, num_idxs_reg=ncols * 128,
                    elem_size=128, single_packet=False, queue_num=q())
                nc.gpsimd.dma_gather(
                    out_ap=gj[:, 0:ncols, :], in_ap=HT[bj:, :],
                    idxs_ap=lpj_t[:, 0:ncols * 8],
                    num_idxs=ncols * 128, num_idxs_reg=ncols * 128,
                    elem_size=128, single_packet=False, queue_num=q())
                if DEBUG and col0 == 0:
                    gif = sbw.tile([128, 8, 128], F32, tag="gif")
                    nc.vector.tensor_copy(gif[:, 0:ncols, :], gi[:, 0:ncols, :])
                    nc.sync.dma_start(dbg_gi[:, 0:ncols * 128],
                                      gif[:, 0:ncols, :].rearrange("p a d -> p (a d)"))
                    gjf = sbw.tile([128, 8, 128], F32, tag="gjf")
                    nc.vector.tensor_copy(gjf[:, 0:ncols, :], gj[:, 0:ncols, :])
                    nc.sync.dma_start(dbg_gj[:, 0:ncols * 128],
                                      gjf[:, 0:ncols, :].rearrange("p a d -> p (a d)"))
                prod = sbg.tile([128, 8, H2], BF16, tag="prod")
                nc.vector.tensor_tensor(out=prod[:, 0:ncols, :],
                                        in0=gi[:, 0:ncols, H2:128],
                                        in1=gj[:, 0:ncols, H2:128],
                                        op=AluOp.mult)
                rr = sbw.tile([128, 8], F32, tag="rr")
                nc.vector.reduce_sum(rr[:, 0:ncols], prod[:, 0:ncols, :],
                                     axis=AxX)
                nc.sync.dma_start(res_out[:, col0:col0 + ncols], rr[:, 0:ncols])

    nc.compile()
    return nc


# =================== top-level kernel ===================

_CACHE = {}


def _get_weight_inputs(cfg, W1, b1, W2, b2, W_attr, b_attr, W_att, b_att):
    W2p = np.zeros((128, cfg.H2), np.float32)
    W2p[:cfg.H1] = np.asarray(W2, np.float32)
    Wcat = np.concatenate([np.asarray(W_attr, np.float32),
                           np.asarray(W_att, np.float32)], axis=1)
    bcat = np.concatenate([np.asarray(b_attr, np.float32),
                           np.asarray(b_att, np.float32)])
    return {
        "W1": np.asarray(W1, np.float32),
        "W2": W2p.astype(ml_dtypes.bfloat16),
        "Wcat": Wcat,
        "b1_rep": np.tile(np.asarray(b1, np.float32)[None, :], (128, 1)),
        "b2_rep": np.tile(np.asarray(b2, np.float32)[None, :], (128, 1)),
        "bcat_rep": np.tile(bcat[None, :], (128, 1)),
    }


def run(cfg, inputs, verbose=True):
    import time
    t0 = time.time()
    in_maps, meta = preprocess(
        cfg, inputs["x"], inputs["train_pos_edge_index"], inputs["edge_index"],
        inputs["edge_weight"], inputs["pos_edge_index"], inputs["neg_edge_index"])
    if verbose:
        print(f"[kernel] preprocess: {time.time()-t0:.1f}s", flush=True)

    wk = _get_weight_inputs(cfg, inputs["W1"], inputs["b1"], inputs["W2"],
                            inputs["b2"], inputs["W_attr"], inputs["b_attr"],
                            inputs["W_att"], inputs["b_att"])
    for im in in_maps:
        im.update(wk)

    key = (cfg.N, tuple(map(tuple, meta["capsA"])), tuple(map(tuple, meta["capsB"])),
           meta["lp_tot"])
    if key not in _CACHE:
        t0 = time.time()
        _CACHE[key] = build_program(cfg, meta["capsA"], meta["capsB"],
                                    meta["lp_tot"], meta["lp_info"])
        if verbose:
            print(f"[kernel] build: {time.time()-t0:.1f}s", flush=True)
    nc = _CACHE[key]

    t0 = time.time()
    res = run_bass_kernel_spmd(nc, in_maps, list(range(cfg.NC)))
    if verbose:
        print(f"[kernel] run: {time.time()-t0:.1f}s", flush=True)
    if res.exec_time_ns is not None:
        print(f"HW exec time: {res.exec_time_ns} ns")
    return postprocess(cfg, meta, res.results)


def postprocess(cfg, meta, results):
    N, NSLICE, NP = cfg.N, cfg.NSLICE, cfg.NP
    attr = np.zeros((N, cfg.NCLS), np.float32)
    att = np.zeros((N, cfg.NCLS), np.float32)
    for c in range(cfg.NC):
        rows = np.asarray(results[c]["attr_att"], np.float32)
        order = meta["ordB"][c]
        lo = c * NSLICE
        attr[lo + order] = rows[:NSLICE, :cfg.NCLS]
        att[lo + order] = rows[:NSLICE, cfg.NCLS:]
    LPB = cfg.LP_PER_CORE
    res_full = np.zeros(cfg.NC * LPB, np.float32)
    for c in range(cfg.NC):
        r = np.asarray(results[c]["res"], np.float32)
        flat = r.T.reshape(-1)     # slot i = col*128 + p -> flat[i]
        perm = meta["lp_perm"][c]
        m = perm >= 0
        res_full[c * LPB + perm[m]] = flat[m]
    return res_full, attr, att


def kernel(x, train_pos_edge_index, edge_index, edge_weight, pos_edge_index,
           neg_edge_index, W1, b1, W2, b2, W_attr, b_attr, W_att, b_att):
    cfg = CFG_FULL
    inputs = dict(x=x, train_pos_edge_index=train_pos_edge_index,
                  edge_index=edge_index, edge_weight=edge_weight,
                  pos_edge_index=pos_edge_index, neg_edge_index=neg_edge_index,
                  W1=W1, b1=b1, W2=W2, b2=b2, W_attr=W_attr, b_attr=b_attr,
                  W_att=W_att, b_att=b_att)
    return run(cfg, inputs)
